# revision 17
# baseline (speedup 1.0000x reference)
"""Trainium2 Bass kernel for nn_Decoder_RNN_Gumbel (8-core SPMD, data-parallel over batch).

Architecture (per core, batch shard of 8):
  - All large operands resident in SBUF: masked enc_key^T (bf16), enc_value (bf16),
    LSTM weights W^T (bf16), dense weights (bf16). Gumbel noise precomputed on host
    (exact fp32, it dominates the argmax) and streamed per-iteration.
  - Feature-major layout: activations stored transposed [feature, batch] so
    per-step tensors put features on SBUF partitions, and activations are the tiny
    N=8 streaming operand of every matmul while weights ride the FWL port.
  - Sigmoids are folded into tanh (weights pre-scaled 0.5 on host, cell state kept
    as 2c, hidden state as 2h with consumer weights pre-halved) so the scalar
    engine needs only the exp_and_others LUT set -> zero ACT_TABLE_LOAD thrash.
  - T=300 sequential steps, hardware For_i loop over iterations of U unrolled
    steps; outputs staged in SBUF rings, DMA'd per iteration, host finishes with
    cheap transposes.
"""

import os
import sys

import numpy as np

try:
    import concourse.bass as bass
except ImportError:  # fresh grading dir: concourse ships with the container image
    for _p in ("/opt/trn_rl_repo", "/root/.axon_site/_ro/trn_rl_repo"):
        if os.path.isdir(_p) and _p not in sys.path:
            sys.path.insert(0, _p)
    import concourse.bass as bass

import ml_dtypes
import concourse.tile as tile
from concourse import bacc, mybir
from concourse.bass_utils import run_bass_kernel_spmd

F32 = mybir.dt.float32
BF16 = mybir.dt.bfloat16
U32 = mybir.dt.uint32
AF = mybir.ActivationFunctionType
OP = mybir.AluOpType

B, L, KV, H, E, T, V, MH = 64, 1024, 256, 512, 256, 300, 33, 512
SOS = 0
NEG_SLOPE = 0.9
EPS = 1e-10
NC = 8          # cores
BL = B // NC    # batch per core = 8
LC = L // 128   # L chunks = 8
KC = KV // 128  # KV chunks = 2
HC = H // 128   # H chunks = 4

_cache = {}


def _build(t_total: int, u: int, use_for_i: bool):
    """Build + compile the per-core Bass program."""
    key = (t_total, u, use_for_i)
    if key in _cache:
        return _cache[key]
    assert t_total % u == 0
    n_iter = t_total // u

    nc = bacc.Bacc("TRN2", target_bir_lowering=False, debug=False,
                   enable_asserts=False, num_devices=NC)

    # ---- DRAM parameters (per-core shard shapes) ----
    ekt = nc.dram_tensor("ekt", [BL, KC, 128, L], BF16, kind="ExternalInput").ap()
    ev = nc.dram_tensor("ev", [BL, LC, 128, KV], BF16, kind="ExternalInput").ap()
    wt = [nc.dram_tensor(f"wt{l}", [8, 128, 4 * H], BF16, kind="ExternalInput").ap()
          for l in range(3)]
    wfc = nc.dram_tensor("wfc", [HC, 128, KV], BF16, kind="ExternalInput").ap()
    w1 = nc.dram_tensor("w1", [4, 128, MH], BF16, kind="ExternalInput").ap()
    w2 = nc.dram_tensor("w2", [4, 128, V], BF16, kind="ExternalInput").ap()
    wemb = nc.dram_tensor("wemb", [V, E], BF16, kind="ExternalInput").ap()
    x0 = nc.dram_tensor("x0", [KC, 128, BL], BF16, kind="ExternalInput").ap()
    gum = nc.dram_tensor("gum", [V, BL, t_total], F32, kind="ExternalInput").ap()
    eye128 = nc.dram_tensor("eye128", [128, 128], F32, kind="ExternalInput").ap()

    # staged layouts: yh [v][t][b], att [t][b][l] bf16, lbl [b][t*8] (host finishes)
    yh = nc.dram_tensor("yh", [V, t_total, BL], F32, kind="ExternalOutput").ap()
    att = nc.dram_tensor("att", [BL, LC, t_total, 128], BF16, kind="ExternalOutput").ap()

    from contextlib import ExitStack
    with tile.TileContext(nc) as tc, ExitStack() as _ctx:
        res = _ctx.enter_context(tc.tile_pool(name="res", bufs=1))
        ring = _ctx.enter_context(tc.tile_pool(name="ring", bufs=2))
        scr = _ctx.enter_context(tc.tile_pool(name="scr", bufs=3))
        pg = _ctx.enter_context(tc.tile_pool(name="pg", bufs=2, space="PSUM"))
        ps = _ctx.enter_context(tc.tile_pool(name="ps", bufs=4, space="PSUM"))
        psy = _ctx.enter_context(tc.tile_pool(name="psy", bufs=2, space="PSUM"))
        if True:
            # ---- resident tiles ----
            ekt_sb = res.tile([128, BL, KC, L], BF16)       # 32KB/p
            ev_sb = res.tile([128, BL, LC, KV], BF16)       # 32KB/p
            wt_sb = [res.tile([128, 8, 4 * H], BF16, tag=f"wt{l}", name=f"wt{l}_sb")
                     for l in range(3)]                     # 96KB/p
            wfc_sb = res.tile([128, HC, KV], BF16)
            w1_sb = res.tile([128, 4, MH], BF16)
            w2_sb = res.tile([128, 4, V], BF16)
            wemb_sb = res.tile([V, E], BF16)
            eye128_sb = res.tile([128, 128], F32)
            ones_p = res.tile([1, 128], F32)     # K=1 lhsT for partition-bcast
            ones_v = res.tile([V, 1], BF16)      # K=V lhsT for partition-sum
            ones_l = res.tile([128, 1], F32)     # K=128 lhsT for partition-sum

            for b in range(BL):
                nc.sync.dma_start(ekt_sb[:, b], ekt[b].rearrange("c p l -> p c l"))
                nc.sync.dma_start(ev_sb[:, b], ev[b].rearrange("c p k -> p c k"))
            for l in range(3):
                nc.sync.dma_start(wt_sb[l][:], wt[l].rearrange("c p m -> p c m"))
            nc.sync.dma_start(wfc_sb[:], wfc.rearrange("c p m -> p c m"))
            nc.sync.dma_start(w1_sb[:], w1.rearrange("c p m -> p c m"))
            nc.sync.dma_start(w2_sb[:], w2.rearrange("c p m -> p c m"))
            nc.sync.dma_start(wemb_sb[:], wemb[:])
            nc.sync.dma_start(eye128_sb[:], eye128[:])
            nc.vector.memset(ones_p[:], 1.0)
            nc.vector.memset(ones_v[:], 1.0)
            nc.vector.memset(ones_l[:], 1.0)

            # ---- state tiles (feature-major, persistent) ----
            # hbf holds 2h (consumer weights pre-halved); cs holds 2c.
            hbf = [res.tile([128, HC, BL], BF16, tag=f"h{l}", name=f"h{l}_sb")
                   for l in range(3)]
            cs = [res.tile([128, HC, BL], F32, tag=f"c{l}", name=f"c{l}_sb")
                  for l in range(3)]
            embbf = res.tile([128, KC, BL], BF16)
            ctxbf = res.tile([128, KC, BL], BF16)
            for l in range(3):
                nc.vector.memset(hbf[l][:], 0.0)
                nc.vector.memset(cs[l][:], 0.0)
            nc.vector.memset(ctxbf[:], 0.0)
            nc.sync.dma_start(embbf[:], x0.rearrange("c p b -> p c b"))
            gum_all = res.tile([V, BL * t_total], F32)
            y_all = res.tile([V, t_total * BL], F32)
            nc.sync.dma_start(gum_all[:], gum.rearrange("v b t -> v (b t)"))
            gum_v = gum_all[:].rearrange("v (b t) -> v b t", b=BL)

            def step(iv, j, att_ring, prev_tail):
                """Emit one decode step; j = index within the iteration (0..u-1)."""
                # --- 3-layer LSTM (all-tanh formulation) ---
                for l in range(3):
                    if l == 0:
                        rhs = [embbf[:, 0], embbf[:, 1], ctxbf[:, 0], ctxbf[:, 1],
                               hbf[0][:, 0], hbf[0][:, 1], hbf[0][:, 2], hbf[0][:, 3]]
                        korder = [4, 5, 6, 7, 2, 3, 0, 1]  # own-h first, emb last
                    else:
                        rhs = [hbf[l - 1][:, k] for k in range(4)] + \
                              [hbf[l][:, k] for k in range(4)]
                        korder = [4, 5, 6, 7, 0, 1, 2, 3]
                    g_ps = pg.tile([128, 16 * BL], F32, tag="gates", name="g_ps")
                    for ki, k in enumerate(korder):
                        if l == 0 and ki == 6 and prev_tail is not None:
                            # previous step's feedback matmuls slot in here so the
                            # PE isn't head-of-line blocked on them while idle
                            prev_tail()
                            prev_tail = None
                        for m in range(16):
                            nc.tensor.matmul(
                                g_ps[:, m * BL:(m + 1) * BL],
                                wt_sb[l][:, k, m * 128:(m + 1) * 128],
                                rhs[k],
                                start=(ki == 0), stop=(ki == 7),
                                skip_group_check=True)
                    if l == 0 and prev_tail is not None:
                        prev_tail()
                        prev_tail = None
                    # cols: [ti(0:32) tf(32:64) g~(64:96) to(96:128)], all tanh
                    tall = scr.tile([128, 16 * BL], F32, tag="tall", name="tall")
                    nc.scalar.activation(tall[:], g_ps[:], AF.Tanh)
                    ti, tf_ = tall[:, 0:4 * BL], tall[:, 4 * BL:8 * BL]
                    gg, to = tall[:, 8 * BL:12 * BL], tall[:, 12 * BL:16 * BL]
                    sflat = cs[l][:].rearrange("p c b -> p (c b)")
                    pp = scr.tile([128, 4 * BL], F32, tag="pp", name="pp")
                    qq = scr.tile([128, 4 * BL], F32, tag="qq", name="qq")
                    # S_new = 0.5*(tf+1)*S + (ti+1)*g~   (S = 2c)
                    nc.vector.scalar_tensor_tensor(pp[:], tf_, 1.0, sflat,
                                                   OP.add, OP.mult)
                    nc.vector.scalar_tensor_tensor(qq[:], ti, 1.0, gg,
                                                   OP.add, OP.mult)
                    nc.vector.scalar_tensor_tensor(sflat, pp[:], 0.5, qq[:],
                                                   OP.mult, OP.add)
                    tc2 = scr.tile([128, 4 * BL], F32, tag="tc2", name="tc2")
                    nc.scalar.activation(tc2[:], sflat, AF.Tanh, scale=0.5)
                    # hbf = 2h = (to+1)*tanh(c)
                    nc.vector.scalar_tensor_tensor(
                        hbf[l][:].rearrange("p c b -> p (c b)"), to, 1.0, tc2[:],
                        OP.add, OP.mult)

                # --- query = h2 @ Wfc^T  (feature-major: [KV, b]) ---
                q_ps = ps.tile([128, KC * BL], F32, tag="sm", name="q_ps")
                qbf = scr.tile([128, KC * BL], BF16, tag="qbf", name="qbf")
                for c in range(KC):
                    for k in range(HC):
                        nc.tensor.matmul(q_ps[:, c * BL:(c + 1) * BL],
                                         wfc_sb[:, k, c * 128:(c + 1) * 128],
                                         hbf[2][:, k],
                                         start=(k == 0), stop=(k == HC - 1))
                    nc.vector.tensor_copy(qbf[:, c * BL:(c + 1) * BL],
                                          q_ps[:, c * BL:(c + 1) * BL])
                qv = qbf[:].rearrange("p (c b) -> p c b", c=KC)

                # --- energy + softmax + ctx, two pipelined half-batches ---
                en_ps = ps.tile([128, BL * LC], F32, tag="sm", name="en_ps")
                ex = scr.tile([128, BL * LC], F32, tag="ex", name="ex")
                sums_ps = psy.tile([1, BL * LC], F32, tag="yv", name="sums_ps")
                s8 = scr.tile([1, BL], F32, tag="s8", name="s8")
                sinv = scr.tile([1, BL], F32, tag="sinv", name="sinv")
                bc_ps = ps.tile([128, BL * LC], F32, tag="sm", name="bc_ps")
                attf = scr.tile([128, BL * LC], F32, tag="attf", name="attf")
                attw = scr.tile([128, BL * LC], BF16, tag="attw", name="attw")
                ctx_ps = ps.tile([128, KC * BL], F32, tag="sm", name="ctx_ps")
                HB = BL // 2
                halves = [(h2, range(h2 * HB, (h2 + 1) * HB),
                           slice(h2 * HB * LC, (h2 + 1) * HB * LC))
                          for h2 in range(2)]
                for h2, bs, sl in halves:
                    for b in bs:
                        for lc in range(LC):
                            for c in range(KC):
                                nc.tensor.matmul(
                                    en_ps[:, b * LC + lc: b * LC + lc + 1],
                                    ekt_sb[:, b, c, lc * 128:(lc + 1) * 128],
                                    qv[:, c, b:b + 1],
                                    start=(c == 0), stop=(c == KC - 1))
                    nc.scalar.activation(ex[:, sl], en_ps[:, sl], AF.Exp)
                for h2, bs, sl in halves:
                    hsl = slice(h2 * HB, (h2 + 1) * HB)
                    nc.tensor.matmul(sums_ps[:, sl], ones_l[:], ex[:, sl],
                                     start=True, stop=True)
                    nc.vector.reduce_sum(
                        s8[:, hsl],
                        sums_ps[:, sl].rearrange("p (b l) -> p b l", b=HB),
                        axis=mybir.AxisListType.X)
                    nc.vector.reciprocal(sinv[:, hsl], s8[:, hsl])
                for h2, bs, sl in halves:
                    hsl = slice(h2 * HB, (h2 + 1) * HB)
                    nc.tensor.matmul(
                        bc_ps[:, sl], ones_p[:],
                        sinv[:, hsl].unsqueeze(2).to_broadcast((1, HB, LC)),
                        start=True, stop=True)
                    nc.vector.tensor_mul(attf[:, sl], ex[:, sl], bc_ps[:, sl])
                    nc.vector.tensor_copy(attw[:, sl], attf[:, sl])
                    for b in bs:
                        for c in range(KC):
                            for lc in range(LC):
                                nc.tensor.matmul(
                                    ctx_ps[:, c * BL + b: c * BL + b + 1],
                                    ev_sb[:, b, lc, c * 128:(c + 1) * 128],
                                    attw[:, b * LC + lc: b * LC + lc + 1],
                                    start=(lc == 0), stop=(lc == LC - 1))
                nc.vector.tensor_copy(ctxbf[:].rearrange("p c b -> p (c b)"),
                                      ctx_ps[:])
                # stage transposed attention rows (bf16)
                at_ps = ps.tile([BL * LC, 128], F32, tag="sm", name="at_ps")
                nc.tensor.transpose(at_ps[:], attf[:], eye128_sb[:])
                nc.scalar.activation(
                    att_ring[:, j * 128:(j + 1) * 128], at_ps[:], AF.Copy)

                # --- mlp1 + LeakyReLU(0.9): 0.9x + 0.1 relu(x) ---
                hm_ps = ps.tile([128, HC * BL], F32, tag="sm", name="hm_ps")
                cat = [qv[:, 0], qv[:, 1], ctxbf[:, 0], ctxbf[:, 1]]
                for k in range(4):
                    for m in range(4):
                        nc.tensor.matmul(hm_ps[:, m * BL:(m + 1) * BL],
                                         w1_sb[:, k, m * 128:(m + 1) * 128],
                                         cat[k],
                                         start=(k == 0), stop=(k == 3),
                                         skip_group_check=True)
                r01 = scr.tile([128, HC * BL], F32, tag="r01", name="r01")
                nc.scalar.activation(r01[:], hm_ps[:], AF.Relu,
                                     scale=1.0 - NEG_SLOPE)
                hmbf = scr.tile([128, HC * BL], BF16, tag="hmbf", name="hmbf")
                nc.vector.scalar_tensor_tensor(
                    hmbf[:], hm_ps[:], NEG_SLOPE, r01[:], OP.mult, OP.add)

                # --- y_t = hmid @ W2^T ([V, b]) ; store logits ---
                y_ps = psy.tile([V, BL], F32, tag="yv", name="y_ps")
                for k in range(4):
                    nc.tensor.matmul(y_ps[:], w2_sb[:, k, :],
                                     hmbf[:, k * BL:(k + 1) * BL],
                                     start=(k == 0), stop=(k == 3))
                nc.vector.tensor_copy(y_all[:, bass.ds(iv * BL + j * BL, BL)], y_ps[:])

                # --- gumbel softmax feedback + argmax label ---
                zf = scr.tile([V, BL], F32, tag="zf", name="zf")
                nc.vector.tensor_add(
                    zf[:], y_ps[:], gum_v[:, :, bass.ds(iv + j, 1)].squeeze(2))
                ez = scr.tile([V, BL], BF16, tag="ez", name="ez")
                nc.scalar.activation(ez[:], zf[:], AF.Exp)

                def tail():
                    ss_ps = psy.tile([1, BL], F32, tag="yv", name="ss_ps")
                    nc.tensor.matmul(ss_ps[:], ones_v[:], ez[:],
                                     start=True, stop=True)
                    sinv2 = scr.tile([1, BL], F32, tag="sinv2", name="sinv2")
                    nc.vector.reciprocal(sinv2[:], ss_ps[:])
                    bc2_ps = ps.tile([128, KC * BL], F32, tag="sm", name="bc2_ps")
                    nc.tensor.matmul(
                        bc2_ps[:], ones_p[:],
                        sinv2[:].unsqueeze(1).to_broadcast((1, KC, BL)),
                        start=True, stop=True)
                    emb_ps = ps.tile([128, KC * BL], F32, tag="sm", name="emb_ps")
                    for c in range(KC):
                        nc.tensor.matmul(emb_ps[:, c * BL:(c + 1) * BL],
                                         wemb_sb[:, c * 128:(c + 1) * 128],
                                         ez[:], start=True, stop=True)
                    embf = scr.tile([128, KC * BL], F32, tag="embf", name="embf")
                    nc.scalar.activation(embf[:], emb_ps[:], AF.Copy)
                    nc.vector.tensor_mul(
                        embbf[:].rearrange("p c b -> p (c b)"), embf[:], bc2_ps[:])
                return tail


            def iteration(iv):
                att_ring = ring.tile([BL * LC, u * 128], BF16, tag="att",
                                     name="att_ring")
                ptail = None
                for j in range(u):
                    ptail = step(iv, j, att_ring, ptail)
                ptail()
                # att stage [b][c][(t p)]: one DMA, 64 contiguous runs
                nc.sync.dma_start(
                    att.rearrange("b c t p -> (b c) (t p)")[
                        :, bass.ds(iv * 128, u * 128)],
                    att_ring[:])

            if use_for_i:
                with tc.For_i(0, t_total, u,
                              hint_engines=(mybir.EngineType.PE,)) as iv:
                    iteration(iv)
            else:
                for it in range(n_iter):
                    iteration(it * u)
            nc.sync.dma_start(yh.rearrange("v t b -> v (t b)"), y_all[:])

    nc.compile()
    _cache[key] = nc
    return nc


def _prep(inputs, t_total):
    """Host-side preprocessing: shard, transpose, cast, gumbel transform,
    and the all-tanh weight rescaling (i/f/o gate rows x0.5; h-consumer
    columns x0.5 because the kernel carries 2h)."""
    bf = ml_dtypes.bfloat16
    f32 = {k: np.asarray(v) for k, v in inputs.items()}
    mask = (np.arange(L)[None, :] <
            np.asarray(f32["final_seq_lens"])[:, None]).astype(np.float32)
    ek_m = np.asarray(f32["enc_key"], np.float32) * mask[:, :, None]   # [B, L, KV]
    ekt = np.ascontiguousarray(ek_m.transpose(0, 2, 1)).reshape(
        B, KC, 128, L).astype(bf)
    ev = np.asarray(f32["enc_value"], np.float32).reshape(B, LC, 128, KV).astype(bf)
    wts = []
    for l in range(3):
        wcat = np.concatenate(
            [np.asarray(f32[f"Wih{l}"], np.float32),
             np.asarray(f32[f"Whh{l}"], np.float32)], axis=1).copy()   # [2048, 1024]
        # h inputs arrive as 2h: halve their columns (Whh always; Wih for l>0)
        wcat[:, H:] *= 0.5
        if l > 0:
            wcat[:, :H] *= 0.5
        # i, f, o gate rows halved: tanh(x/2) replaces sigmoid(x)
        wcat[0 * H:1 * H] *= 0.5
        wcat[1 * H:2 * H] *= 0.5
        wcat[3 * H:4 * H] *= 0.5
        wts.append(np.ascontiguousarray(wcat.T).reshape(8, 128, 4 * H).astype(bf))
    wfc_s = np.asarray(f32["W_fc"], np.float32) * 0.5          # h2 arrives as 2h
    wfc = np.ascontiguousarray(wfc_s.T).reshape(HC, 128, KV).astype(bf)
    w1 = np.ascontiguousarray(np.asarray(f32["W_mlp1"], np.float32).T).reshape(
        4, 128, MH).astype(bf)
    w2 = np.ascontiguousarray(np.asarray(f32["W_mlp2"], np.float32).T).reshape(
        4, 128, V).astype(bf)
    wemb = np.ascontiguousarray(np.asarray(f32["W_emb"], np.float32).T).astype(bf)
    sos = (np.asarray(f32["W_emb"], np.float32)[:, SOS] +
           np.asarray(f32["b_emb"], np.float32))                        # [E]
    x0 = np.tile(sos.reshape(KC, 128, 1), (1, 1, BL)).astype(bf)
    gu = np.asarray(f32["gumbel_u"], np.float32)[:t_total]
    gall = -np.log(-np.log(gu + EPS) + EPS)                             # [T, B, V]
    gvbt = np.ascontiguousarray(gall.transpose(2, 1, 0))                # [V, B, T]
    eye128 = np.eye(128, dtype=np.float32)

    for bn in ("b_emb", "b_fc", "b_mlp1", "b_mlp2",
               "bih0", "bhh0", "bih1", "bhh1", "bih2", "bhh2"):
        if np.any(np.asarray(f32[bn]) != 0):
            raise NotImplementedError(f"nonzero bias {bn} not supported")

    in_maps = []
    for c in range(NC):
        s = slice(c * BL, (c + 1) * BL)
        in_maps.append({
            "ekt": np.ascontiguousarray(ekt[s]),
            "ev": np.ascontiguousarray(ev[s]),
            "wt0": wts[0], "wt1": wts[1], "wt2": wts[2],
            "wfc": wfc, "w1": w1, "w2": w2, "wemb": wemb,
            "x0": x0, "eye128": eye128,
            "gum": np.ascontiguousarray(gvbt[:, s, :]),
        })
    return in_maps


def run(inputs, t_total=T, u=10, use_for_i=True, trace=False, trace_kwargs=None):
    nc = _build(t_total, u, use_for_i)
    in_maps = _prep(inputs, t_total)
    res = run_bass_kernel_spmd(nc, in_maps, list(range(NC)), trace=trace,
                               **(trace_kwargs or {}))
    y_hat = np.concatenate(
        [res.results[c]["yh"].transpose(2, 1, 0) for c in range(NC)], 0)
    gu = np.asarray(inputs["gumbel_u"], np.float32)[:t_total]
    gall = -np.log(-np.log(gu + EPS) + EPS)             # [T, B, V]
    y_lbl = np.ascontiguousarray(
        (y_hat + gall.transpose(1, 0, 2)).argmax(2).astype(np.int32))
    attn = np.concatenate(
        [np.asarray(res.results[c]["att"], np.float32).swapaxes(2, 3)
            .reshape(BL, L, t_total) for c in range(NC)], 0)
    lab = np.asarray(inputs["labels_padded"]).T.copy()
    return (y_hat, y_lbl, lab, attn), res


def kernel(**inputs):
    outs, _ = run(inputs)
    return outs


# revision 19
# speedup vs baseline: 1.0109x; 1.0109x over previous
"""Trainium2 Bass kernel for nn_Decoder_RNN_Gumbel (8-core SPMD, data-parallel over batch).

Architecture (per core, batch shard of 8):
  - All large operands resident in SBUF: masked enc_key^T (bf16), enc_value (bf16),
    LSTM weights W^T (bf16), dense weights (bf16). Gumbel noise precomputed on host
    (exact fp32, it dominates the argmax) and streamed per-iteration.
  - Feature-major layout: activations stored transposed [feature, batch] so
    per-step tensors put features on SBUF partitions, and activations are the tiny
    N=8 streaming operand of every matmul while weights ride the FWL port.
  - Sigmoids are folded into tanh (weights pre-scaled 0.5 on host, cell state kept
    as 2c, hidden state as 2h with consumer weights pre-halved) so the scalar
    engine needs only the exp_and_others LUT set -> zero ACT_TABLE_LOAD thrash.
  - T=300 sequential steps, hardware For_i loop over iterations of U unrolled
    steps; outputs staged in SBUF rings, DMA'd per iteration, host finishes with
    cheap transposes.
"""

import os
import sys

import numpy as np

try:
    import concourse.bass as bass
except ImportError:  # fresh grading dir: concourse ships with the container image
    for _p in ("/opt/trn_rl_repo", "/root/.axon_site/_ro/trn_rl_repo"):
        if os.path.isdir(_p) and _p not in sys.path:
            sys.path.insert(0, _p)
    import concourse.bass as bass

import ml_dtypes
import concourse.tile as tile
from concourse import bacc, mybir
from concourse.bass_utils import run_bass_kernel_spmd

F32 = mybir.dt.float32
BF16 = mybir.dt.bfloat16
U32 = mybir.dt.uint32
AF = mybir.ActivationFunctionType
OP = mybir.AluOpType

B, L, KV, H, E, T, V, MH = 64, 1024, 256, 512, 256, 300, 33, 512
SOS = 0
NEG_SLOPE = 0.9
EPS = 1e-10
NC = 8          # cores
BL = B // NC    # batch per core = 8
LC = L // 128   # L chunks = 8
KC = KV // 128  # KV chunks = 2
HC = H // 128   # H chunks = 4

_cache = {}


def _build(t_total: int, u: int, use_for_i: bool):
    """Build + compile the per-core Bass program."""
    key = (t_total, u, use_for_i)
    if key in _cache:
        return _cache[key]
    assert t_total % u == 0
    n_iter = t_total // u

    nc = bacc.Bacc("TRN2", target_bir_lowering=False, debug=False,
                   enable_asserts=False, num_devices=NC)

    # ---- DRAM parameters (per-core shard shapes) ----
    ekt = nc.dram_tensor("ekt", [BL, KC, 128, L], BF16, kind="ExternalInput").ap()
    ev = nc.dram_tensor("ev", [BL, LC, 128, KV], BF16, kind="ExternalInput").ap()
    wt = [nc.dram_tensor(f"wt{l}", [8, 128, 4 * H], BF16, kind="ExternalInput").ap()
          for l in range(3)]
    wfc = nc.dram_tensor("wfc", [HC, 128, KV], BF16, kind="ExternalInput").ap()
    w1 = nc.dram_tensor("w1", [4, 128, MH], BF16, kind="ExternalInput").ap()
    w2 = nc.dram_tensor("w2", [4, 128, V], BF16, kind="ExternalInput").ap()
    wemb = nc.dram_tensor("wemb", [V, E], BF16, kind="ExternalInput").ap()
    x0 = nc.dram_tensor("x0", [KC, 128, BL], BF16, kind="ExternalInput").ap()
    gum = nc.dram_tensor("gum", [V, BL, t_total], F32, kind="ExternalInput").ap()
    eye128 = nc.dram_tensor("eye128", [128, 128], F32, kind="ExternalInput").ap()

    # staged layouts: yh [v][t][b], att [t][b][l] bf16, lbl [b][t*8] (host finishes)
    yh = nc.dram_tensor("yh", [V, t_total, BL], F32, kind="ExternalOutput").ap()
    att = nc.dram_tensor("att", [BL, LC, t_total, 128], BF16, kind="ExternalOutput").ap()

    from contextlib import ExitStack
    with tile.TileContext(nc) as tc, ExitStack() as _ctx:
        res = _ctx.enter_context(tc.tile_pool(name="res", bufs=1))
        ring = _ctx.enter_context(tc.tile_pool(name="ring", bufs=2))
        scr = _ctx.enter_context(tc.tile_pool(name="scr", bufs=3))
        pg = _ctx.enter_context(tc.tile_pool(name="pg", bufs=3, space="PSUM"))
        ps = _ctx.enter_context(tc.tile_pool(name="ps", bufs=3, space="PSUM"))
        psy = _ctx.enter_context(tc.tile_pool(name="psy", bufs=2, space="PSUM"))
        if True:
            # ---- resident tiles ----
            ekt_sb = res.tile([128, BL, KC, L], BF16)       # 32KB/p
            ev_sb = res.tile([128, BL, LC, KV], BF16)       # 32KB/p
            wt_sb = [res.tile([128, 8, 4 * H], BF16, tag=f"wt{l}", name=f"wt{l}_sb")
                     for l in range(3)]                     # 96KB/p
            wfc_sb = res.tile([128, HC, KV], BF16)
            w1_sb = res.tile([128, 4, MH], BF16)
            w2_sb = res.tile([128, 4, V], BF16)
            wemb_sb = res.tile([V, E], BF16)
            eye128_sb = res.tile([128, 128], F32)
            ones_p = res.tile([1, 128], F32)     # K=1 lhsT for partition-bcast
            ones_v = res.tile([V, 1], BF16)      # K=V lhsT for partition-sum
            ones_l = res.tile([128, 1], F32)     # K=128 lhsT for partition-sum

            for b in range(BL):
                nc.sync.dma_start(ekt_sb[:, b], ekt[b].rearrange("c p l -> p c l"))
                nc.sync.dma_start(ev_sb[:, b], ev[b].rearrange("c p k -> p c k"))
            for l in range(3):
                nc.sync.dma_start(wt_sb[l][:], wt[l].rearrange("c p m -> p c m"))
            nc.sync.dma_start(wfc_sb[:], wfc.rearrange("c p m -> p c m"))
            nc.sync.dma_start(w1_sb[:], w1.rearrange("c p m -> p c m"))
            nc.sync.dma_start(w2_sb[:], w2.rearrange("c p m -> p c m"))
            nc.sync.dma_start(wemb_sb[:], wemb[:])
            nc.sync.dma_start(eye128_sb[:], eye128[:])
            nc.vector.memset(ones_p[:], 1.0)
            nc.vector.memset(ones_v[:], 1.0)
            nc.vector.memset(ones_l[:], 1.0)

            # ---- state tiles (feature-major, persistent) ----
            # hbf holds 2h (consumer weights pre-halved); cs holds 2c.
            hbf = [res.tile([128, HC, BL], BF16, tag=f"h{l}", name=f"h{l}_sb")
                   for l in range(3)]
            cs = [res.tile([128, HC, BL], F32, tag=f"c{l}", name=f"c{l}_sb")
                  for l in range(3)]
            embbf = res.tile([128, KC, BL], BF16)
            ctxbf = res.tile([128, KC, BL], BF16)
            for l in range(3):
                nc.vector.memset(hbf[l][:], 0.0)
                nc.vector.memset(cs[l][:], 0.0)
            nc.vector.memset(ctxbf[:], 0.0)
            nc.sync.dma_start(embbf[:], x0.rearrange("c p b -> p c b"))
            gum_all = res.tile([V, BL * t_total], F32)
            y_all = res.tile([V, t_total * BL], F32)
            nc.sync.dma_start(gum_all[:], gum.rearrange("v b t -> v (b t)"))
            gum_v = gum_all[:].rearrange("v (b t) -> v b t", b=BL)

            # ---- LSTM emission helpers (all-tanh formulation) ----
            def lstm_rhs(l):
                if l == 0:
                    return [embbf[:, 0], embbf[:, 1], ctxbf[:, 0], ctxbf[:, 1],
                            hbf[0][:, 0], hbf[0][:, 1], hbf[0][:, 2], hbf[0][:, 3]]
                return [hbf[l - 1][:, k] for k in range(4)] + \
                       [hbf[l][:, k] for k in range(4)]

            def gmm(l, g_ps, ks, start=False, stop=False):
                rhs = lstm_rhs(l)
                for ki, k in enumerate(ks):
                    for m in range(16):
                        nc.tensor.matmul(
                            g_ps[:, m * BL:(m + 1) * BL],
                            wt_sb[l][:, k, m * 128:(m + 1) * 128],
                            rhs[k],
                            start=(start and ki == 0),
                            stop=(stop and ki == len(ks) - 1),
                            skip_group_check=True)

            def gchain(l, g_ps):
                # cols: [ti(0:32) tf(32:64) g~(64:96) to(96:128)], all tanh
                tall = scr.tile([128, 16 * BL], F32, tag="tall", name="tall")
                nc.scalar.activation(tall[:], g_ps[:], AF.Tanh)
                ti, tf_ = tall[:, 0:4 * BL], tall[:, 4 * BL:8 * BL]
                gg, to = tall[:, 8 * BL:12 * BL], tall[:, 12 * BL:16 * BL]
                sflat = cs[l][:].rearrange("p c b -> p (c b)")
                pp = scr.tile([128, 4 * BL], F32, tag="pp", name="pp")
                qq = scr.tile([128, 4 * BL], F32, tag="qq", name="qq")
                # S_new = 0.5*(tf+1)*S + (ti+1)*g~   (S = 2c)
                nc.vector.scalar_tensor_tensor(pp[:], tf_, 1.0, sflat,
                                               OP.add, OP.mult)
                nc.vector.scalar_tensor_tensor(qq[:], ti, 1.0, gg,
                                               OP.add, OP.mult)
                nc.vector.scalar_tensor_tensor(sflat, pp[:], 0.5, qq[:],
                                               OP.mult, OP.add)
                tc2 = scr.tile([128, 4 * BL], F32, tag="tc2", name="tc2")
                nc.scalar.activation(tc2[:], sflat, AF.Tanh, scale=0.5)
                # hbf = 2h = (to+1)*tanh(c)
                nc.vector.scalar_tensor_tensor(
                    hbf[l][:].rearrange("p c b -> p (c b)"), to, 1.0, tc2[:],
                    OP.add, OP.mult)

            def lstm_full(l):
                g_ps = pg.tile([128, 16 * BL], F32, tag="gates", name="g_ps")
                gmm(l, g_ps, [4, 5, 6, 7], start=True)
                gmm(l, g_ps, [0, 1, 2, 3] if l else [2, 3, 0, 1], stop=True)
                gchain(l, g_ps)

            HB = BL // 2
            halves = [(h2, range(h2 * HB, (h2 + 1) * HB),
                       slice(h2 * HB * LC, (h2 + 1) * HB * LC))
                      for h2 in range(2)]

            def step(iv, j, att_ring, nxt):
                """Emit logical step j; `nxt` True -> interleave step j+1's
                LSTM matmuls into this step's serial-chain shadows."""
                # --- query = h2 @ Wfc^T  (feature-major: [KV, b]) ---
                q_ps = ps.tile([128, KC * BL], F32, tag="sm", name="q_ps")
                qbf = scr.tile([128, KC * BL], BF16, tag="qbf", name="qbf")
                for c in range(KC):
                    for k in range(HC):
                        nc.tensor.matmul(q_ps[:, c * BL:(c + 1) * BL],
                                         wfc_sb[:, k, c * 128:(c + 1) * 128],
                                         hbf[2][:, k],
                                         start=(k == 0), stop=(k == HC - 1))
                    nc.vector.tensor_copy(qbf[:, c * BL:(c + 1) * BL],
                                          q_ps[:, c * BL:(c + 1) * BL])
                qv = qbf[:].rearrange("p (c b) -> p c b", c=KC)

                # --- energy + softmax(divide) + ctx, pipelined halves ---
                en_ps = ps.tile([128, BL * LC], F32, tag="sm", name="en_ps")
                ex = scr.tile([128, BL * LC], F32, tag="ex", name="ex")
                sums_ps = psy.tile([1, BL * LC], F32, tag="yv", name="sums_ps")
                s8 = scr.tile([1, BL], F32, tag="s8", name="s8")
                sinv = scr.tile([1, BL], F32, tag="sinv", name="sinv")
                bc_ps = ps.tile([128, BL * LC], F32, tag="sm", name="bc_ps")
                attf = scr.tile([128, BL * LC], F32, tag="attf", name="attf")
                attw = scr.tile([128, BL * LC], BF16, tag="attw", name="attw")
                ctx_ps = ps.tile([128, KC * BL], F32, tag="sm", name="ctx_ps")
                for h2, bs, sl in halves:
                    for b in bs:
                        for lc in range(LC):
                            for c in range(KC):
                                nc.tensor.matmul(
                                    en_ps[:, b * LC + lc: b * LC + lc + 1],
                                    ekt_sb[:, b, c, lc * 128:(lc + 1) * 128],
                                    qv[:, c, b:b + 1],
                                    start=(c == 0), stop=(c == KC - 1))
                    nc.scalar.activation(ex[:, sl], en_ps[:, sl], AF.Exp)
                for h2, bs, sl in halves:
                    hsl = slice(h2 * HB, (h2 + 1) * HB)
                    nc.tensor.matmul(sums_ps[:, sl], ones_l[:], ex[:, sl],
                                     start=True, stop=True)
                    nc.vector.reduce_sum(
                        s8[:, hsl],
                        sums_ps[:, sl].rearrange("p (b l) -> p b l", b=HB),
                        axis=mybir.AxisListType.X)
                    nc.vector.reciprocal(sinv[:, hsl], s8[:, hsl])
                for h2, bs, sl in halves:
                    hsl = slice(h2 * HB, (h2 + 1) * HB)
                    nc.tensor.matmul(
                        bc_ps[:, sl], ones_p[:],
                        sinv[:, hsl].unsqueeze(2).to_broadcast((1, HB, LC)),
                        start=True, stop=True)
                    nc.vector.tensor_mul(attf[:, sl], ex[:, sl], bc_ps[:, sl])
                    nc.vector.tensor_copy(attw[:, sl], attf[:, sl])
                for h2, bs, sl in halves:
                    for b in bs:
                        for c in range(KC):
                            for lc in range(LC):
                                nc.tensor.matmul(
                                    ctx_ps[:, c * BL + b: c * BL + b + 1],
                                    ev_sb[:, b, lc, c * 128:(c + 1) * 128],
                                    attw[:, b * LC + lc: b * LC + lc + 1],
                                    start=(lc == 0), stop=(lc == LC - 1))
                nc.vector.tensor_copy(ctxbf[:].rearrange("p c b -> p (c b)"),
                                      ctx_ps[:])
                # stage transposed attention rows (bf16)
                at_ps = ps.tile([BL * LC, 128], F32, tag="sm", name="at_ps")
                nc.tensor.transpose(at_ps[:], attf[:], eye128_sb[:])
                nc.scalar.activation(
                    att_ring[:, j * 128:(j + 1) * 128], at_ps[:], AF.Copy)

                # --- mlp1 + LeakyReLU(0.9): 0.9x + 0.1 relu(x) ---
                hm_ps = ps.tile([128, HC * BL], F32, tag="sm", name="hm_ps")
                cat = [qv[:, 0], qv[:, 1], ctxbf[:, 0], ctxbf[:, 1]]
                for k in range(4):
                    for m in range(4):
                        nc.tensor.matmul(hm_ps[:, m * BL:(m + 1) * BL],
                                         w1_sb[:, k, m * 128:(m + 1) * 128],
                                         cat[k],
                                         start=(k == 0), stop=(k == 3),
                                         skip_group_check=True)
                r01 = scr.tile([128, HC * BL], F32, tag="r01", name="r01")
                nc.scalar.activation(r01[:], hm_ps[:], AF.Relu,
                                     scale=1.0 - NEG_SLOPE)
                hmbf = scr.tile([128, HC * BL], BF16, tag="hmbf", name="hmbf")

                # next-step L0 ctx-part fills the relu/hmbf shadow
                if nxt:
                    g0 = pg.tile([128, 16 * BL], F32, tag="gates", name="g_ps")
                    gmm(0, g0, [2, 3], start=True)

                nc.vector.scalar_tensor_tensor(
                    hmbf[:], hm_ps[:], NEG_SLOPE, r01[:], OP.mult, OP.add)

                # --- y_t = hmid @ W2^T ([V, b]) ; logits into resident y_all ---
                y_ps = psy.tile([V, BL], F32, tag="yv", name="y_ps")
                for k in range(4):
                    nc.tensor.matmul(y_ps[:], w2_sb[:, k, :],
                                     hmbf[:, k * BL:(k + 1) * BL],
                                     start=(k == 0), stop=(k == 3))
                nc.vector.tensor_copy(y_all[:, bass.ds(iv * BL + j * BL, BL)],
                                      y_ps[:])
                zf = scr.tile([V, BL], F32, tag="zf", name="zf")
                nc.vector.tensor_add(
                    zf[:], y_ps[:], gum_v[:, :, bass.ds(iv + j, 1)].squeeze(2))
                ez = scr.tile([V, BL], BF16, tag="ez", name="ez")
                nc.scalar.activation(ez[:], zf[:], AF.Exp)

                # next-step L0/L1 own-h matmuls fill the y/z/ez + tail shadows
                if nxt:
                    gmm(0, g0, [4, 5, 6, 7])
                    g1 = pg.tile([128, 16 * BL], F32, tag="gates", name="g_ps")
                    gmm(1, g1, [4, 5, 6, 7], start=True)

                # --- gumbel-softmax feedback (emb = (e^z @ Wemb^T) / sum) ---
                ss_ps = psy.tile([1, BL], F32, tag="yv", name="ss_ps")
                nc.tensor.matmul(ss_ps[:], ones_v[:], ez[:],
                                 start=True, stop=True)
                ssb = scr.tile([1, BL], F32, tag="ssb", name="ssb")
                nc.vector.reciprocal(ssb[:], ss_ps[:])
                bc2_ps = ps.tile([128, KC * BL], F32, tag="sm", name="bc2_ps")
                nc.tensor.matmul(
                    bc2_ps[:], ones_p[:],
                    ssb[:].unsqueeze(1).to_broadcast((1, KC, BL)),
                    start=True, stop=True)
                emb_ps = ps.tile([128, KC * BL], F32, tag="sm", name="emb_ps")
                for c in range(KC):
                    nc.tensor.matmul(emb_ps[:, c * BL:(c + 1) * BL],
                                     wemb_sb[:, c * 128:(c + 1) * 128],
                                     ez[:], start=True, stop=True)
                embf = scr.tile([128, KC * BL], F32, tag="embf", name="embf")
                nc.scalar.activation(embf[:], emb_ps[:], AF.Copy)
                nc.vector.tensor_mul(
                    embbf[:].rearrange("p c b -> p (c b)"), embf[:], bc2_ps[:])

                # finish next-step LSTM in the feedback shadow
                if nxt:
                    gmm(0, g0, [0, 1], stop=True)
                    gchain(0, g0)
                    gmm(1, g1, [0, 1, 2, 3], stop=True)
                    gchain(1, g1)
                    g2 = pg.tile([128, 16 * BL], F32, tag="gates", name="g_ps")
                    gmm(2, g2, [4, 5, 6, 7], start=True)
                    gmm(2, g2, [0, 1, 2, 3], stop=True)
                    gchain(2, g2)

            def iteration(iv):
                att_ring = ring.tile([BL * LC, u * 128], BF16, tag="att",
                                     name="att_ring")
                for l in range(3):
                    lstm_full(l)
                for j in range(u):
                    step(iv, j, att_ring, nxt=(j < u - 1))
                # att stage [b][c][(t p)]: one DMA, 64 contiguous runs
                nc.sync.dma_start(
                    att.rearrange("b c t p -> (b c) (t p)")[
                        :, bass.ds(iv * 128, u * 128)],
                    att_ring[:])

            if use_for_i:
                with tc.For_i(0, t_total, u,
                              hint_engines=(mybir.EngineType.PE,)) as iv:
                    iteration(iv)
            else:
                for it in range(n_iter):
                    iteration(it * u)
            nc.sync.dma_start(yh.rearrange("v t b -> v (t b)"), y_all[:])

    nc.compile()
    _cache[key] = nc
    return nc


def _prep(inputs, t_total):
    """Host-side preprocessing: shard, transpose, cast, gumbel transform,
    and the all-tanh weight rescaling (i/f/o gate rows x0.5; h-consumer
    columns x0.5 because the kernel carries 2h)."""
    bf = ml_dtypes.bfloat16
    f32 = {k: np.asarray(v) for k, v in inputs.items()}
    mask = (np.arange(L)[None, :] <
            np.asarray(f32["final_seq_lens"])[:, None]).astype(np.float32)
    ek_m = np.asarray(f32["enc_key"], np.float32) * mask[:, :, None]   # [B, L, KV]
    ekt = np.ascontiguousarray(ek_m.transpose(0, 2, 1)).reshape(
        B, KC, 128, L).astype(bf)
    ev = np.asarray(f32["enc_value"], np.float32).reshape(B, LC, 128, KV).astype(bf)
    wts = []
    for l in range(3):
        wcat = np.concatenate(
            [np.asarray(f32[f"Wih{l}"], np.float32),
             np.asarray(f32[f"Whh{l}"], np.float32)], axis=1).copy()   # [2048, 1024]
        # h inputs arrive as 2h: halve their columns (Whh always; Wih for l>0)
        wcat[:, H:] *= 0.5
        if l > 0:
            wcat[:, :H] *= 0.5
        # i, f, o gate rows halved: tanh(x/2) replaces sigmoid(x)
        wcat[0 * H:1 * H] *= 0.5
        wcat[1 * H:2 * H] *= 0.5
        wcat[3 * H:4 * H] *= 0.5
        wts.append(np.ascontiguousarray(wcat.T).reshape(8, 128, 4 * H).astype(bf))
    wfc_s = np.asarray(f32["W_fc"], np.float32) * 0.5          # h2 arrives as 2h
    wfc = np.ascontiguousarray(wfc_s.T).reshape(HC, 128, KV).astype(bf)
    w1 = np.ascontiguousarray(np.asarray(f32["W_mlp1"], np.float32).T).reshape(
        4, 128, MH).astype(bf)
    w2 = np.ascontiguousarray(np.asarray(f32["W_mlp2"], np.float32).T).reshape(
        4, 128, V).astype(bf)
    wemb = np.ascontiguousarray(np.asarray(f32["W_emb"], np.float32).T).astype(bf)
    sos = (np.asarray(f32["W_emb"], np.float32)[:, SOS] +
           np.asarray(f32["b_emb"], np.float32))                        # [E]
    x0 = np.tile(sos.reshape(KC, 128, 1), (1, 1, BL)).astype(bf)
    gu = np.asarray(f32["gumbel_u"], np.float32)[:t_total]
    gall = -np.log(-np.log(gu + EPS) + EPS)                             # [T, B, V]
    gvbt = np.ascontiguousarray(gall.transpose(2, 1, 0))                # [V, B, T]
    eye128 = np.eye(128, dtype=np.float32)

    for bn in ("b_emb", "b_fc", "b_mlp1", "b_mlp2",
               "bih0", "bhh0", "bih1", "bhh1", "bih2", "bhh2"):
        if np.any(np.asarray(f32[bn]) != 0):
            raise NotImplementedError(f"nonzero bias {bn} not supported")

    in_maps = []
    for c in range(NC):
        s = slice(c * BL, (c + 1) * BL)
        in_maps.append({
            "ekt": np.ascontiguousarray(ekt[s]),
            "ev": np.ascontiguousarray(ev[s]),
            "wt0": wts[0], "wt1": wts[1], "wt2": wts[2],
            "wfc": wfc, "w1": w1, "w2": w2, "wemb": wemb,
            "x0": x0, "eye128": eye128,
            "gum": np.ascontiguousarray(gvbt[:, s, :]),
        })
    return in_maps


def run(inputs, t_total=T, u=10, use_for_i=True, trace=False, trace_kwargs=None):
    nc = _build(t_total, u, use_for_i)
    in_maps = _prep(inputs, t_total)
    res = run_bass_kernel_spmd(nc, in_maps, list(range(NC)), trace=trace,
                               **(trace_kwargs or {}))
    y_hat = np.concatenate(
        [res.results[c]["yh"].transpose(2, 1, 0) for c in range(NC)], 0)
    gu = np.asarray(inputs["gumbel_u"], np.float32)[:t_total]
    gall = -np.log(-np.log(gu + EPS) + EPS)             # [T, B, V]
    y_lbl = np.ascontiguousarray(
        (y_hat + gall.transpose(1, 0, 2)).argmax(2).astype(np.int32))
    attn = np.concatenate(
        [np.asarray(res.results[c]["att"], np.float32).swapaxes(2, 3)
            .reshape(BL, L, t_total) for c in range(NC)], 0)
    lab = np.asarray(inputs["labels_padded"]).T.copy()
    return (y_hat, y_lbl, lab, attn), res


def kernel(**inputs):
    outs, _ = run(inputs)
    return outs
